# revision 11
# baseline (speedup 1.0000x reference)
"""Multi-head attention (B=2, L=2048, dim=1024, 16 heads) on 8 Trainium2 cores.

Sharding: 8 cores = 2 (batch) x 4 (head groups of 4 heads). Each core runs an
identical Bass program on its own slice (SPMD, no collectives); the host sums
the 4 per-head-group partial projection outputs per batch and adds the bias.

Per-core dataflow (bf16 matmul operands, fp32 PSUM accumulation):
  xT [1024, 2048]  (x[b] transposed, channel-major, bf16)
  V token-major [128 tok, 4 heads, 64+1] (ones column fused for the softmax
    denominator), qT/kT feature-major [128 (2 heads x 64d), 2048]
  ST[k, q] = kT.T @ qT    (K=64 contraction per head)
  PT = exp(ST / 8)        (ScalarE, PSUM -> SBUF bf16; no max-subtraction
                           needed: |S/8| <= ~7 so exp is safely in range)
  OT[d, q] += V.T @ PT    (M=65: row 64 accumulates the softmax denominator)
  OT_norm = OT * bcast(1/denom)   (reciprocal_approx_fast on DVE + DMA
                                   broadcast; even head scaled on DVE, odd
                                   head on GpSimd, DMA moves it to rows 64+)
  out[tok, c] = OT_norm.T @ wpT   (contract 4 heads x 64 channels)

Scheduling: everything is one fused phase. The PE instruction stream is the
attention loop (scores -> exp -> PV per 128-key block) with QKV-projection /
out-projection work units interleaved between key blocks via a static filler
schedule, so the PE never waits on a phase boundary and the Scalar engine's
exp stream (the secondary bottleneck, ~137us) hides under the PE's ~170us.
The last query chunk runs its head pairs in reverse order so the final
epilogue chain overlaps the other pair's attention.
"""

import os
import numpy as np

B, L, C = 2, 2048, 1024
H, D = 16, 64
HL = 4            # heads per core (local)
PAIRS = 2         # head pairs per core
CT = C // 128     # 8 contraction tiles for the projections
TOK = L // 128    # 16 key-token tiles
QW = 512          # query tile width
QS = L // QW      # 4 query tiles
NCORES = 8

_cache = {}


def _build_nc():
    import concourse.bass as bass
    import concourse.mybir as mybir
    import concourse.tile as tile
    from concourse import bacc

    F32 = mybir.dt.float32
    BF16 = mybir.dt.bfloat16
    EXP = mybir.ActivationFunctionType.Exp
    COPY = mybir.ActivationFunctionType.Copy
    LN = mybir.ActivationFunctionType.Ln

    nc = bacc.Bacc("TRN2", target_bir_lowering=False, debug=False,
                   num_devices=NCORES)

    xT = nc.declare_dram_parameter("xT", [C, L], BF16, isOutput=False)
    wT = nc.declare_dram_parameter("wT", [C, 3 * HL * D], BF16, isOutput=False)
    wpT = nc.declare_dram_parameter("wpT", [HL * D, C], BF16, isOutput=False)
    out = nc.declare_dram_parameter("out", [L, C], F32, isOutput=True)

    with tile.TileContext(nc) as tc:
        from contextlib import ExitStack
        with ExitStack() as ctx:
            xpool = ctx.enter_context(tc.tile_pool(name="x", bufs=1))
            wpool = ctx.enter_context(tc.tile_pool(name="w", bufs=1))
            wppool = ctx.enter_context(tc.tile_pool(name="wp", bufs=1))
            qkpool = ctx.enter_context(tc.tile_pool(name="qk", bufs=1))
            vpool = ctx.enter_context(tc.tile_pool(name="v", bufs=1))
            ptpool = ctx.enter_context(tc.tile_pool(name="pt", bufs=8))
            otpool = ctx.enter_context(tc.tile_pool(name="ot", bufs=1))
            obpool = ctx.enter_context(tc.tile_pool(name="ob", bufs=4))
            ocpool = ctx.enter_context(tc.tile_pool(name="oc", bufs=2))
            rpool = ctx.enter_context(tc.tile_pool(name="r", bufs=2))
            psS = ctx.enter_context(tc.tile_pool(name="psS", bufs=2, space="PSUM"))
            psOT = ctx.enter_context(tc.tile_pool(name="psOT", bufs=2, space="PSUM"))
            psF = ctx.enter_context(tc.tile_pool(name="psF", bufs=2, space="PSUM"))

            # ---- input loads: x first (everything needs it), then w/wp ------
            x_t, w_t = [], []
            for i in range(CT):
                tx = xpool.tile([128, L], BF16, name=f"x{i}", tag=f"x{i}")
                nc.sync.dma_start(out=tx[:, 0:L // 2],
                                  in_=xT[128 * i:128 * (i + 1), 0:L // 2])
                nc.sync.dma_start(out=tx[:, L // 2:L],
                                  in_=xT[128 * i:128 * (i + 1), L // 2:L])
                x_t.append(tx)
            for i in range(CT):
                tw = wpool.tile([128, 3 * HL * D], BF16, name=f"w{i}", tag=f"w{i}")
                nc.sync.dma_start(out=tw, in_=wT[128 * i:128 * (i + 1), :])
                w_t.append(tw)
            wp_t = []
            for p in range(PAIRS):
                t = wppool.tile([128, C], BF16, name=f"wp{p}", tag=f"wp{p}")
                nc.sync.dma_start(out=t, in_=wpT[2 * D * p:2 * D * (p + 1), :])
                wp_t.append(t)

            ones_s = vpool.tile([128, HL, 1], F32, name="ones_s", tag="ones_s")
            nc.vector.memset(ones_s, 1.0)

            # persistent SBUF tensors
            v_t = [None] * TOK
            qk_t = {}
            for p in range(PAIRS):
                for nm in ("q", "k"):
                    qk_t[(nm, p)] = qkpool.tile(
                        [128, L], BF16, name=f"{nm}{p}", tag=f"{nm}{p}")
            # per-(pair, qs) tiles: the tile dependency tracker is
            # coarse-grained, so a single [128, L] tile would serialize the
            # out-projection of chunk qs-1 behind the epilogue of chunk qs
            ot_sb = [[otpool.tile([128, QW], BF16, name=f"otp{p}q{qs}",
                                  tag=f"otp{p}q{qs}")
                      for qs in range(QS)] for p in range(PAIRS)]

            # ---- work units -------------------------------------------------
            def unit_v(t):
                ps = psF.tile([128, HL * D], F32, name="psv", tag="ps")
                for c in range(CT):
                    nc.tensor.matmul(
                        ps,
                        lhsT=x_t[c][:, 128 * t:128 * (t + 1)],
                        rhs=w_t[c][:, 2 * HL * D:3 * HL * D],
                        start=(c == 0), stop=(c == CT - 1),
                    )
                vt = vpool.tile([128, HL, D + 1], BF16, name=f"v{t}", tag=f"v{t}")
                nc.vector.tensor_copy(out=vt[:, :, D:D + 1], in_=ones_s)
                nc.vector.tensor_copy(
                    out=vt[:, :, 0:D],
                    in_=ps.rearrange("p (h d) -> p h d", h=HL),
                )
                v_t[t] = vt

            def unit_qk(nm, p, ns):
                j = 0 if nm == "q" else 1
                ps = psF.tile([128, QW], F32, name="psqk", tag="ps")
                for c in range(CT):
                    nc.tensor.matmul(
                        ps,
                        lhsT=w_t[c][:, j * HL * D + 128 * p:
                                    j * HL * D + 128 * (p + 1)],
                        rhs=x_t[c][:, QW * ns:QW * (ns + 1)],
                        start=(c == 0), stop=(c == CT - 1),
                    )
                nc.vector.tensor_copy(
                    out=qk_t[(nm, p)][:, QW * ns:QW * (ns + 1)], in_=ps)

            def unit_proj(qs, tt, nh, last=False):
                t = 4 * qs + tt
                ps = psF.tile([128, QW], F32, name="psp", tag="ps")
                for p2 in range(PAIRS):
                    nc.tensor.matmul(
                        ps,
                        lhsT=ot_sb[p2][qs][:, 128 * tt:128 * (tt + 1)],
                        rhs=wp_t[p2][:, QW * nh:QW * (nh + 1)],
                        start=(p2 == 0), stop=(p2 == PAIRS - 1),
                    )
                ob = obpool.tile([128, QW], F32, name="ob", tag="ob")
                if last:
                    nc.scalar.activation(out=ob, in_=ps, func=COPY)
                else:
                    nc.vector.tensor_copy(out=ob, in_=ps)
                nc.sync.dma_start(
                    out=out[128 * t:128 * (t + 1), QW * nh:QW * (nh + 1)],
                    in_=ob)

            # ---- static filler schedule ------------------------------------
            # fillers[(qs, p, kb)] -> list of closures emitted before that
            # attention iteration's score matmuls.
            fillers = {}

            def addf(qs, p, kb, fn):
                fillers.setdefault((qs, p, kb), []).append(fn)

            # (0,0): V7..V15 just-in-time (Vt needed by PV at kb=t), K/Q next
            v_slots = {0: 6, 1: 7, 3: 8, 4: 9, 5: 10, 7: 11, 8: 12, 9: 13,
                       11: 14, 12: 15}
            for kb, t in v_slots.items():
                addf(0, 0, kb, (lambda t=t: unit_v(t)))
            addf(0, 0, 2, lambda: unit_qk("k", 0, 1))
            addf(0, 0, 6, lambda: unit_qk("k", 0, 2))
            addf(0, 0, 10, lambda: unit_qk("k", 0, 3))
            addf(0, 0, 13, lambda: unit_qk("k", 1, 1))
            addf(0, 0, 14, lambda: unit_qk("q", 0, 1))
            addf(0, 1, 0, lambda: unit_qk("k", 1, 2))
            addf(0, 1, 2, lambda: unit_qk("k", 1, 3))
            addf(0, 1, 4, lambda: unit_qk("q", 1, 1))
            # proj(qs-1) spread over qs's iterations (start at kb>=9 so the
            # previous epilogue's DVE reciprocal chain (~8.6us) has drained)
            for qs in (1, 2):
                for u in range(8):
                    tt, nh = divmod(u, 2)
                    pr, kb = (0, 9 + u) if u < 5 else (1, 2 * (u - 5))
                    addf(qs, pr, kb, (lambda qs=qs, tt=tt, nh=nh:
                                      unit_proj(qs - 1, tt, nh)))
            addf(1, 0, 7, lambda: unit_qk("q", 0, 2))
            addf(1, 1, 7, lambda: unit_qk("q", 1, 2))
            addf(2, 0, 7, lambda: unit_qk("q", 1, 3))
            addf(2, 1, 7, lambda: unit_qk("q", 0, 3))
            # qs=3 runs pairs in order (1, 0); proj(2) spread over it
            for u in range(8):
                tt, nh = divmod(u, 2)
                pr, kb = (1, 9 + u) if u < 5 else (0, 2 * (u - 5))
                addf(3, pr, kb, (lambda tt=tt, nh=nh: unit_proj(2, tt, nh)))

            # ---- attention + epilogue --------------------------------------
            def epilogue(qs, p, ot_a, ot_b):
                # 1/denom = exp(-ln(denom)) on the Scalar engine: ln and exp
                # share one activation table, the ln reads the PSUM row
                # directly, and the slow DVE reciprocal (6.5us per pair of
                # head-of-line DVE queue time) disappears entirely.
                lns = rpool.tile([65, 2 * QW], F32, name="lns", tag="lns")
                nc.scalar.activation(
                    out=lns[64:65, 0:QW], in_=ot_a[64:65, :], func=LN)
                nc.scalar.activation(
                    out=lns[64:65, QW:2 * QW], in_=ot_b[64:65, :], func=LN)
                oc = ocpool.tile([65, 2 * QW], F32, name="oc", tag="oc")
                nc.vector.tensor_copy(out=oc[:, 0:QW], in_=ot_a)
                nc.vector.tensor_copy(out=oc[:, QW:2 * QW], in_=ot_b)
                rsb = rpool.tile([65, 2 * QW], F32, name="rsb", tag="rsb")
                nc.scalar.activation(
                    out=rsb[64:65, :], in_=lns[64:65, :], func=EXP, scale=-1.0)
                pstep = rsb.ap[0][0]
                rbc_a = rpool.tile([64, QW], F32, name="rbca", tag="rbca")
                nc.sync.dma_start(out=rbc_a, in_=bass.AP(
                    tensor=rsb.tensor, offset=rsb.offset + 64 * pstep,
                    ap=[[pstep, 1], [0, 64], [1, QW]]))
                rbc_b = rpool.tile([64, QW], F32, name="rbcb", tag="rbcb")
                nc.sync.dma_start(out=rbc_b, in_=bass.AP(
                    tensor=rsb.tensor, offset=rsb.offset + 64 * pstep + QW,
                    ap=[[pstep, 1], [0, 64], [1, QW]]))
                # even head normalized on DVE, odd head on GpSimd (parallel),
                # DMA shifts the odd head into partitions 64-127
                nc.vector.tensor_mul(
                    out=ot_sb[p][qs][0:64, :],
                    in0=oc[0:64, 0:QW], in1=rbc_a)
                stg = rpool.tile([64, QW], BF16, name="stg", tag="stg")
                nc.gpsimd.tensor_mul(
                    out=stg, in0=oc[0:64, QW:2 * QW], in1=rbc_b)
                nc.sync.dma_start(
                    out=ot_sb[p][qs][64:128, :], in_=stg)

            def attention(qs, p):
                kT = qk_t[("k", p)]
                qT = qk_t[("q", p)]
                ot_a = psOT.tile([65, QW], F32, name="ot_a", tag="ot")
                ot_b = psOT.tile([65, QW], F32, name="ot_b", tag="ot")
                for kb in range(TOK):
                    for fn in fillers.get((qs, p, kb), ()):
                        fn()
                    st = psS.tile([128, 2 * QW], F32, name="st", tag="st")
                    nc.tensor.matmul(
                        st[:, 0:QW],
                        lhsT=kT[0:64, 128 * kb:128 * (kb + 1)],
                        rhs=qT[0:64, QW * qs:QW * (qs + 1)],
                        start=True, stop=True,
                    )
                    nc.tensor.matmul(
                        st[:, QW:2 * QW],
                        lhsT=kT[64:128, 128 * kb:128 * (kb + 1)],
                        rhs=qT[64:128, QW * qs:QW * (qs + 1)],
                        start=True, stop=True,
                    )
                    pt = ptpool.tile([128, 2 * QW], BF16, name="pt", tag="pt")
                    nc.scalar.activation(out=pt, in_=st, func=EXP, scale=0.125)
                    nc.tensor.matmul(
                        ot_a,
                        lhsT=v_t[kb][:, 2 * p, :],
                        rhs=pt[:, 0:QW],
                        start=(kb == 0), stop=(kb == TOK - 1),
                    )
                    nc.tensor.matmul(
                        ot_b,
                        lhsT=v_t[kb][:, 2 * p + 1, :],
                        rhs=pt[:, QW:2 * QW],
                        start=(kb == 0), stop=(kb == TOK - 1),
                    )
                epilogue(qs, p, ot_a, ot_b)

            # ---- prologue: enough V/K/Q for the first pair-sweep -----------
            for t in range(6):
                unit_v(t)
            unit_qk("k", 0, 0)
            unit_qk("q", 0, 0)
            unit_qk("k", 1, 0)
            unit_qk("q", 1, 0)

            # ---- main loop --------------------------------------------------
            for qs in range(QS):
                order = (1, 0) if qs == QS - 1 else (0, 1)
                for p in order:
                    attention(qs, p)

            # tail: out-projection of the last query chunk (ACT copies; the
            # Scalar engine is idle once the last exp has drained)
            for u in range(8):
                tt, nh = divmod(u, 2)
                unit_proj(QS - 1, tt, nh, last=True)

    nc.compile()
    return nc


def _get_nc():
    if "nc" not in _cache:
        _cache["nc"] = _build_nc()
    return _cache["nc"]


def kernel(x, w_qkv, w_proj, b_proj):
    import ml_dtypes
    from concourse.bass_utils import run_bass_kernel_spmd

    x = np.asarray(x, dtype=np.float32)
    w_qkv = np.asarray(w_qkv, dtype=np.float32)
    w_proj = np.asarray(w_proj, dtype=np.float32)
    b_proj = np.asarray(b_proj, dtype=np.float32)

    nc = _get_nc()
    in_maps = []
    for core in range(NCORES):
        b, g = divmod(core, 4)
        rows = np.concatenate([
            np.arange(C * j + HL * D * g, C * j + HL * D * (g + 1))
            for j in range(3)
        ])
        in_maps.append({
            "xT": np.ascontiguousarray(x[b].T).astype(ml_dtypes.bfloat16),
            "wT": np.ascontiguousarray(w_qkv[rows].T).astype(ml_dtypes.bfloat16),
            "wpT": np.ascontiguousarray(
                w_proj[:, HL * D * g:HL * D * (g + 1)].T).astype(ml_dtypes.bfloat16),
        })

    res = run_bass_kernel_spmd(
        nc, in_maps, list(range(NCORES)),
        trace=bool(os.environ.get("KERNEL_TRACE")),
    )
    _cache["last_results"] = res

    out = np.empty((B, L, C), dtype=np.float32)
    for b in range(B):
        acc = res.results[4 * b]["out"].astype(np.float32)
        for g in range(1, 4):
            acc = acc + res.results[4 * b + g]["out"]
        out[b] = acc + b_proj[None, :]
    return out
